# revision 2
# baseline (speedup 1.0000x reference)
"""Trainium2 Bass kernel for nn_BATransform (8-core data-parallel over batch).

Pipeline per core (one sample, c-major layout (C, T*H*W)):
  conv1(1x1x1)+BN+ReLU -> adaptive pools -> conv_p/q/t + softmax ->
  spatial block-mix transforms (P over h-blocks, Q over w-blocks, TM over
  t-halves) applied as per-partition-scalar DVE ops -> conv2(1x1x1)+BN+ReLU.

BN folded into conv weights host-side.  Matmuls run in bf16 (PE full rate);
spatial transform in bf16 with fp32 per-channel scalar APs.
"""

import numpy as np
import ml_dtypes

# Problem constants (hardcoded per contract)
NB, C, T, H, W = 8, 512, 8, 32, 32
S, TS, K, TK = 4, 2, 8, 8
EPS = 1e-5
POS = T * H * W        # 8192
HB, WB, TB = H // S, W // S, T // TS   # 8, 8, 4
G = C // K             # 64 channels per k-group
NCORES = 8
NCHUNK = 4             # channel chunks of 128
CPK = 128              # channels per chunk

_CACHE = {}


def _split_waits(nc, mybir, max_waits=1):
    """This walrus build accepts at most one sync-wait per instruction;
    hoist extras onto same-engine NoOps placed just before."""
    for f in nc.m.functions:
        for bb in f.blocks:
            out = []
            for inst in bb.instructions:
                si = inst.sync_info
                if si is not None and si.on_wait and len(si.on_wait) > max_waits:
                    waits = list(si.on_wait)
                    keep = waits[-max_waits:]
                    extra = waits[:-max_waits]
                    k = 0
                    while extra:
                        chunk, extra = extra[:max_waits], extra[max_waits:]
                        nop = mybir.InstNoOp(
                            name=f"{inst.name}-ws{k}",
                            sync_info=mybir.SyncInfo(on_wait=chunk, on_update=[]),
                        )
                        nop.engine = inst.engine
                        nc.register_instruction(nop)
                        out.append(nop)
                        k += 1
                    inst.sync_info = mybir.SyncInfo(
                        on_wait=keep, on_update=list(si.on_update)
                    )
                out.append(inst)
            bb.instructions = out


def _build():
    import concourse.bass as bass
    import concourse.tile as tile
    import concourse.mybir as mybir

    f32 = mybir.dt.float32
    bf16 = mybir.dt.bfloat16
    AF = mybir.ActivationFunctionType
    ALU = mybir.AluOpType
    AX = mybir.AxisListType

    nc = bass.Bass("TRN2", target_bir_lowering=False, debug=False,
                   num_devices=NCORES)

    x_d = nc.dram_tensor("x", [C, POS], f32, kind="ExternalInput")
    w1t_d = nc.dram_tensor("w1t", [C, K], bf16, kind="ExternalInput")
    b1_d = nc.dram_tensor("b1", [K, 1], f32, kind="ExternalInput")
    wpt_d = nc.dram_tensor("wpt", [K, 4 * 128], f32, kind="ExternalInput")
    bp_d = nc.dram_tensor("bp", [T, 128], f32, kind="ExternalInput")
    wqt_d = nc.dram_tensor("wqt", [K, 4 * 128], f32, kind="ExternalInput")
    bq_d = nc.dram_tensor("bq", [T, 128], f32, kind="ExternalInput")
    wtt_d = nc.dram_tensor("wtt", [K, 2 * 32], f32, kind="ExternalInput")
    bt_d = nc.dram_tensor("bt", [1, 32], f32, kind="ExternalInput")
    w2t_d = nc.dram_tensor("w2t", [C, C], bf16, kind="ExternalInput")
    b2_d = nc.dram_tensor("b2", [C, 1], f32, kind="ExternalInput")
    y_d = nc.dram_tensor("y", [C, POS], f32, kind="ExternalOutput")

    with tile.TileContext(nc) as tc:
        with (
            tc.tile_pool(name="big", bufs=5) as big,        # bf16 (128, POS)
            tc.tile_pool(name="stage", bufs=2) as stage,    # f32 (128, POS//2)
            tc.tile_pool(name="consts", bufs=1) as consts,
            tc.tile_pool(name="small", bufs=1) as small,
            tc.tile_pool(name="sm2", bufs=4) as sm2,
            tc.tile_pool(name="tmp", bufs=4) as tmpp,       # (128, 256) bf16
            tc.tile_pool(name="tmb", bufs=2) as tmbp,       # (128, POS//2) bf16
            tc.tile_pool(name="outp", bufs=3) as outp,      # (128, 512) f32
            tc.tile_pool(name="psum", bufs=8, space="PSUM") as psp,
            tc.tile_pool(name="dram", bufs=1, space="DRAM") as dramp,
        ):
            # ---- weights / consts ----
            w1t = []
            for q in range(NCHUNK):
                t_ = consts.tile([CPK, K], bf16, tag=f"w1t{q}")
                nc.sync.dma_start(t_, x_d[:] if False else w1t_d[:][q * CPK:(q + 1) * CPK, :])
                w1t.append(t_)
            b1 = consts.tile([K, 1], f32, tag="b1")
            nc.sync.dma_start(b1, b1_d[:])
            wpt = consts.tile([K, 4 * 128], f32, tag="wpt")
            nc.sync.dma_start(wpt, wpt_d[:])
            bp = consts.tile([T, 128], f32, tag="bp")
            nc.sync.dma_start(bp, bp_d[:])
            wqt = consts.tile([K, 4 * 128], f32, tag="wqt")
            nc.sync.dma_start(wqt, wqt_d[:])
            bq = consts.tile([T, 128], f32, tag="bq")
            nc.sync.dma_start(bq, bq_d[:])
            wtt = consts.tile([K, 2 * 32], f32, tag="wtt")
            nc.sync.dma_start(wtt, wtt_d[:])
            bt = consts.tile([1, 32], f32, tag="bt")
            nc.sync.dma_start(bt, bt_d[:])
            w2t = []
            for q in range(NCHUNK):
                t_ = consts.tile([CPK, C], bf16, tag=f"w2t{q}")
                nc.sync.dma_start(t_, w2t_d[:][q * CPK:(q + 1) * CPK, :])
                w2t.append(t_)
            b2 = []
            for oc in range(NCHUNK):
                t_ = consts.tile([CPK, 1], f32, tag=f"b2_{oc}")
                nc.sync.dma_start(t_, b2_d[:][oc * CPK:(oc + 1) * CPK, :])
                b2.append(t_)

            # ---- load x, convert to bf16 ----
            xbf = []
            for q in range(NCHUNK):
                xb = big.tile([CPK, POS], bf16, tag="big")
                for h in range(2):
                    xf = stage.tile([CPK, POS // 2], f32, tag="stage")
                    nc.sync.dma_start(
                        xf, x_d[:][q * CPK:(q + 1) * CPK,
                                   h * (POS // 2):(h + 1) * (POS // 2)])
                    nc.vector.tensor_copy(
                        xb[:, h * (POS // 2):(h + 1) * (POS // 2)], xf)
                xbf.append(xb)

            # ---- conv1 + BN + ReLU -> o (K, POS) bf16 ----
            o_sb = small.tile([K, POS], bf16, tag="o")
            for nt in range(16):
                ps = psp.tile([K, 512], f32, tag="ps")
                for q in range(NCHUNK):
                    nc.tensor.matmul(
                        ps, w1t[q], xbf[q][:, nt * 512:(nt + 1) * 512],
                        start=(q == 0), stop=(q == NCHUNK - 1))
                nc.scalar.activation(
                    o_sb[:, nt * 512:(nt + 1) * 512], ps, AF.Relu,
                    bias=b1, scale=1.0)

            # ---- pools ----
            rp = sm2.tile([K, T * S], f32, tag="rp")   # (8, 32) [t,a]
            nc.vector.tensor_reduce(
                rp, o_sb.rearrange("p (g r) -> p g r", r=HB * W),
                AX.X, ALU.max)
            cp = sm2.tile([K, T * S], f32, tag="cp")   # (8, 32) [t,b]
            nc.vector.tensor_reduce(
                cp, o_sb.rearrange("p (t h b v) -> p t b h v", t=T, h=H, b=S),
                AX.XY, ALU.max)
            tp = sm2.tile([K, TS], f32, tag="tp")      # (8, 2)
            nc.vector.tensor_reduce(
                tp, o_sb.rearrange("p (g r) -> p g r", g=TS),
                AX.X, ALU.add)

            # ---- p/q/tm small matmuls + softmax (fp32) ----
            def softmax_rowgroups(name, ps_tile, bias_tile, parts, ngroups, gsz):
                """exp/sum/recip/mult over contiguous groups of gsz."""
                raw = sm2.tile([parts, ngroups * gsz], f32, tag=f"{name}_raw")
                nc.vector.tensor_tensor(raw, ps_tile, bias_tile, ALU.add)
                ex = sm2.tile([parts, ngroups * gsz], f32, tag=f"{name}_ex")
                nc.scalar.activation(ex, raw, AF.Exp)
                ss = sm2.tile([parts, ngroups], f32, tag=f"{name}_ss")
                nc.vector.tensor_reduce(
                    ss, ex.rearrange("p (g r) -> p g r", r=gsz), AX.X, ALU.add)
                rr = sm2.tile([parts, ngroups], f32, tag=f"{name}_rr")
                nc.vector.reciprocal(rr, ss)
                sm = sm2.tile([parts, ngroups * gsz], f32, tag=f"{name}_sm")
                for b in range(gsz):
                    nc.vector.tensor_tensor(
                        sm.rearrange("p (g r) -> p g r", r=gsz)[:, :, b],
                        ex.rearrange("p (g r) -> p g r", r=gsz)[:, :, b],
                        rr, ALU.mult)
                return sm

            ps_p = psp.tile([T, 128], f32, tag="ps")
            for i in range(4):
                nc.tensor.matmul(
                    ps_p, rp.rearrange("p (t a) -> p a t", a=S)[:, i, :],
                    wpt[:, i * 128:(i + 1) * 128],
                    start=(i == 0), stop=(i == 3))
            p_sm = softmax_rowgroups("p", ps_p, bp, T, 32, 4)

            ps_q = psp.tile([T, 128], f32, tag="ps")
            for i in range(4):
                nc.tensor.matmul(
                    ps_q, cp.rearrange("p (t b) -> p b t", b=S)[:, i, :],
                    wqt[:, i * 128:(i + 1) * 128],
                    start=(i == 0), stop=(i == 3))
            q_sm = softmax_rowgroups("q", ps_q, bq, T, 32, 4)

            ps_t = psp.tile([1, 32], f32, tag="ps")
            for i in range(2):
                nc.tensor.matmul(
                    ps_t, tp[:, i:i + 1], wtt[:, i * 32:(i + 1) * 32],
                    start=(i == 0), stop=(i == 1))
            t_sm = softmax_rowgroups("t", ps_t, bt, 1, 16, 2)

            # ---- round-trip through DRAM to build per-channel scalar tables ----
            p_scr = dramp.tile([T, 128], f32, tag="p_scr")
            q_scr = dramp.tile([T, 128], f32, tag="q_scr")
            t_scr = dramp.tile([1, 32], f32, tag="t_scr")
            nc.gpsimd.dma_start(p_scr, p_sm)
            nc.gpsimd.dma_start(q_scr, q_sm)
            nc.gpsimd.dma_start(t_scr, t_sm)

            def bcast_tbl(name, scr, ncols, q):
                """(128, T*ncols or ncols) table: row c <- values for k-group
                of channel q*128+c. scr is DRAM (T, 128)[t, k*16+j] for p/q
                (ncols=16), or (1, 32)[k*4+j] for tm (ncols=4)."""
                per_t = scr.shape[0] == T
                width = (T if per_t else 1) * ncols
                tbl = consts.tile([CPK, width], f32, tag=f"{name}{q}")
                for half in range(2):
                    kk = 2 * q + half
                    if per_t:
                        in_ap = bass.AP(
                            tensor=tbl.tensor if False else scr.tensor,
                            offset=scr.offset + kk * ncols,
                            ap=[[0, 64], [128, T], [1, ncols]])
                        out_ap = tbl[half * 64:(half + 1) * 64, :].rearrange(
                            "p (t g) -> p t g", g=ncols)
                    else:
                        in_ap = bass.AP(
                            tensor=scr.tensor,
                            offset=scr.offset + kk * ncols,
                            ap=[[0, 64], [1, ncols]])
                        out_ap = tbl[half * 64:(half + 1) * 64, :]
                    nc.gpsimd.dma_start(out_ap, in_ap)
                return tbl

            # ---- spatial transform per chunk (bf16) ----
            ytm = []
            for q in range(NCHUNK):
                P_bc = bcast_tbl("Pbc", p_scr, 16, q)   # (128, [t,a,b])
                Q_bc = bcast_tbl("Qbc", q_scr, 16, q)   # (128, [t,b,a])
                T_bc = bcast_tbl("Tbc", t_scr, 4, q)    # (128, [a2,b2])

                xq = xbf[q]
                y1 = big.tile([CPK, POS], bf16, tag="big")
                # Q-pass: mix w-blocks (stride-8 groups of v within each h)
                for t in range(T):
                    for bw in range(S):
                        def qblk(src, aw):
                            return bass.AP(
                                tensor=src.tensor,
                                offset=src.offset + t * 1024 + aw * WB,
                                ap=[list(src.ap[0]), [W, H], [1, WB]])
                        acc = None
                        for aw in range(S):
                            scal = Q_bc[:, t * 16 + bw * 4 + aw:
                                        t * 16 + bw * 4 + aw + 1]
                            if aw == 0:
                                acc = tmpp.tile([CPK, H, WB], bf16, tag="acc")
                                nc.vector.tensor_scalar_mul(
                                    acc, qblk(xq, 0), scal)
                            elif aw < S - 1:
                                nacc = tmpp.tile([CPK, H, WB], bf16, tag="acc")
                                nc.vector.scalar_tensor_tensor(
                                    nacc, qblk(xq, aw), scal, acc,
                                    ALU.mult, ALU.add)
                                acc = nacc
                            else:
                                nc.vector.scalar_tensor_tensor(
                                    qblk(y1, bw), qblk(xq, aw), scal, acc,
                                    ALU.mult, ALU.add)
                # P-pass: mix h-blocks (contiguous 256-blocks), y1 -> xq buffer
                for t in range(T):
                    for ah in range(S):
                        def pblk(src, b):
                            return src[:, t * 1024 + b * 256:
                                       t * 1024 + b * 256 + 256]
                        acc = None
                        for bh in range(S):
                            scal = P_bc[:, t * 16 + ah * 4 + bh:
                                        t * 16 + ah * 4 + bh + 1]
                            if bh == 0:
                                acc = tmpp.tile([CPK, 256], bf16, tag="acc2")
                                nc.vector.tensor_scalar_mul(
                                    acc, pblk(y1, 0), scal)
                            elif bh < S - 1:
                                nacc = tmpp.tile([CPK, 256], bf16, tag="acc2")
                                nc.vector.scalar_tensor_tensor(
                                    nacc, pblk(y1, bh), scal, acc,
                                    ALU.mult, ALU.add)
                                acc = nacc
                            else:
                                nc.vector.scalar_tensor_tensor(
                                    pblk(xq, ah), pblk(y1, bh), scal, acc,
                                    ALU.mult, ALU.add)
                # TM-pass: mix t-halves, xq -> y1 (y1 becomes y_tm)
                HALF = POS // 2
                for a2 in range(TS):
                    tmb = tmbp.tile([CPK, HALF], bf16, tag="tmb")
                    nc.vector.tensor_scalar_mul(
                        tmb, xq[:, 0:HALF], T_bc[:, a2 * 2:a2 * 2 + 1])
                    nc.vector.scalar_tensor_tensor(
                        y1[:, a2 * HALF:(a2 + 1) * HALF],
                        xq[:, HALF:POS], T_bc[:, a2 * 2 + 1:a2 * 2 + 2],
                        tmb, ALU.mult, ALU.add)
                ytm.append(y1)

            # ---- conv2 + BN + ReLU -> y ----
            for oc in range(NCHUNK):
                for ntg in range(2):
                    pss = [psp.tile([CPK, 512], f32, tag="ps",
                                    name=f"ps_c2_{oc}_{ntg}_{j}")
                           for j in range(8)]
                    for cq in range(NCHUNK):
                        for j in range(8):
                            nt = ntg * 8 + j
                            nc.tensor.matmul(
                                pss[j],
                                w2t[cq][:, oc * CPK:(oc + 1) * CPK],
                                ytm[cq][:, nt * 512:(nt + 1) * 512],
                                start=(cq == 0), stop=(cq == NCHUNK - 1))
                    for j in range(8):
                        nt = ntg * 8 + j
                        ob = outp.tile([CPK, 512], f32, tag="ob")
                        nc.scalar.activation(ob, pss[j], AF.Relu,
                                             bias=b2[oc], scale=1.0)
                        nc.sync.dma_start(
                            y_d[:][oc * CPK:(oc + 1) * CPK,
                                   nt * 512:(nt + 1) * 512], ob)

    _split_waits(nc, mybir)
    return nc


def _host_prep(inputs):
    """Fold BN into conv weights, build device-layout weight arrays."""
    f = np.float32
    conv1_w = np.asarray(inputs["conv1_w"], f)
    conv1_b = np.asarray(inputs["conv1_b"], f)
    s1 = np.asarray(inputs["bn1_g"], f) / np.sqrt(np.asarray(inputs["bn1_v"], f) + EPS)
    w1 = conv1_w * s1[:, None]
    b1 = (conv1_b - np.asarray(inputs["bn1_m"], f)) * s1 + np.asarray(inputs["bn1_b"], f)

    convp_w = np.asarray(inputs["convp_w"], f)   # (128, 8, 4)
    convq_w = np.asarray(inputs["convq_w"], f)
    convt_w = np.asarray(inputs["convt_w"], f)   # (32, 8, 2)

    wpt = np.transpose(convp_w, (1, 2, 0)).reshape(K, 4 * 128).copy()
    bp = np.tile(np.asarray(inputs["convp_b"], f)[None, :], (T, 1))

    # permute q outputs o=k*16+a*4+b -> o'=k*16+b*4+a  (a innermost for softmax)
    perm = np.arange(128).reshape(8, 4, 4).transpose(0, 2, 1).reshape(128)
    wq_p = convq_w[perm]                          # row o' = orig row (k,a,b)
    bq_p = np.asarray(inputs["convq_b"], f)[perm]
    wqt = np.transpose(wq_p, (1, 2, 0)).reshape(K, 4 * 128).copy()
    bq = np.tile(bq_p[None, :], (T, 1))

    wtt = (np.transpose(convt_w, (1, 2, 0)) / (TB * H * W)).reshape(K, 2 * 32).copy()
    bt = np.asarray(inputs["convt_b"], f).reshape(1, 32)

    conv2_w = np.asarray(inputs["conv2_w"], f)
    s2 = np.asarray(inputs["bn2_g"], f) / np.sqrt(np.asarray(inputs["bn2_v"], f) + EPS)
    w2 = conv2_w * s2[:, None]
    b2 = (np.asarray(inputs["conv2_b"], f) - np.asarray(inputs["bn2_m"], f)) * s2 \
        + np.asarray(inputs["bn2_b"], f)

    bf = ml_dtypes.bfloat16
    return {
        "w1t": np.ascontiguousarray(w1.T).astype(bf),
        "b1": b1.reshape(K, 1).copy(),
        "wpt": wpt, "bp": bp.copy(),
        "wqt": wqt, "bq": bq.copy(),
        "wtt": wtt, "bt": bt.copy(),
        "w2t": np.ascontiguousarray(w2.T).astype(bf),
        "b2": b2.reshape(C, 1).copy(),
    }


def kernel(**inputs) -> np.ndarray:
    from concourse.bass_utils import run_bass_kernel_spmd

    if "nc" not in _CACHE:
        _CACHE["nc"] = _build()
    nc = _CACHE["nc"]

    shared = _host_prep(inputs)
    x = np.asarray(inputs["x"], np.float32)       # (8, 512, 8, 32, 32)
    in_maps = []
    for i in range(NCORES):
        m = dict(shared)
        m["x"] = np.ascontiguousarray(x[i].reshape(C, POS))
        in_maps.append(m)

    res = run_bass_kernel_spmd(nc, in_maps, list(range(NCORES)))
    out = np.stack([res.results[i]["y"].reshape(C, T, H, W)
                    for i in range(NCORES)])
    return out.astype(np.float32)


# revision 6
# speedup vs baseline: 1795.6828x; 1795.6828x over previous
"""Trainium2 Bass kernel for nn_BATransform (8-core data-parallel over batch).

Pipeline per core (one sample, c-major layout (C, T*H*W)):
  conv1(1x1x1)+BN+ReLU -> adaptive pools -> conv_p/q/t + softmax ->
  spatial block-mix transforms (P over h-blocks, Q over w-blocks, TM over
  t-halves) applied as per-partition-scalar DVE ops -> conv2(1x1x1)+BN+ReLU.

BN folded into conv weights host-side.  Matmuls run in bf16 (PE full rate);
spatial transform in bf16 with fp32 per-channel scalar APs.
"""

import numpy as np
import ml_dtypes

# Problem constants (hardcoded per contract)
NB, C, T, H, W = 8, 512, 8, 32, 32
S, TS, K, TK = 4, 2, 8, 8
EPS = 1e-5
POS = T * H * W        # 8192
HB, WB, TB = H // S, W // S, T // TS   # 8, 8, 4
G = C // K             # 64 channels per k-group
NCORES = 8
NCHUNK = 4             # channel chunks of 128
CPK = 128              # channels per chunk

_CACHE = {}


def _split_waits(nc, mybir, max_waits=1):
    """This walrus build accepts at most one sync-wait per instruction;
    hoist extras onto same-engine NoOps placed just before."""
    for f in nc.m.functions:
        for bb in f.blocks:
            out = []
            for inst in bb.instructions:
                si = inst.sync_info
                if si is not None and si.on_wait and len(si.on_wait) > max_waits:
                    waits = list(si.on_wait)
                    keep = waits[-max_waits:]
                    extra = waits[:-max_waits]
                    k = 0
                    while extra:
                        chunk, extra = extra[:max_waits], extra[max_waits:]
                        nop = mybir.InstNoOp(
                            name=f"{inst.name}-ws{k}",
                            sync_info=mybir.SyncInfo(on_wait=chunk, on_update=[]),
                        )
                        nop.engine = inst.engine
                        nc.register_instruction(nop)
                        out.append(nop)
                        k += 1
                    inst.sync_info = mybir.SyncInfo(
                        on_wait=keep, on_update=list(si.on_update)
                    )
                out.append(inst)
            bb.instructions = out


def _build():
    import concourse.bass as bass
    import concourse.tile as tile
    import concourse.mybir as mybir

    f32 = mybir.dt.float32
    bf16 = mybir.dt.bfloat16
    AF = mybir.ActivationFunctionType
    ALU = mybir.AluOpType
    AX = mybir.AxisListType

    nc = bass.Bass("TRN2", target_bir_lowering=False, debug=False,
                   num_devices=NCORES)

    x_d = nc.dram_tensor("x", [C, POS], f32, kind="ExternalInput")
    w1t_d = nc.dram_tensor("w1t", [C, K], bf16, kind="ExternalInput")
    b1_d = nc.dram_tensor("b1", [K, 1], f32, kind="ExternalInput")
    wpt_d = nc.dram_tensor("wpt", [K, 4 * 128], f32, kind="ExternalInput")
    bp_d = nc.dram_tensor("bp", [T, 128], f32, kind="ExternalInput")
    wqt_d = nc.dram_tensor("wqt", [K, 4 * 128], f32, kind="ExternalInput")
    bq_d = nc.dram_tensor("bq", [T, 128], f32, kind="ExternalInput")
    wtt_d = nc.dram_tensor("wtt", [K, 2 * 32], f32, kind="ExternalInput")
    bt_d = nc.dram_tensor("bt", [1, 32], f32, kind="ExternalInput")
    w2t_d = nc.dram_tensor("w2t", [C, C], bf16, kind="ExternalInput")
    b2_d = nc.dram_tensor("b2", [C, 1], f32, kind="ExternalInput")
    y_d = nc.dram_tensor("y", [C, POS], f32, kind="ExternalOutput")

    with tile.TileContext(nc) as tc:
        with (
            tc.tile_pool(name="big", bufs=5) as big,        # bf16 (128, POS)
            tc.tile_pool(name="stage", bufs=2) as stage,    # f32 (128, POS//2)
            tc.tile_pool(name="consts", bufs=1) as consts,
            tc.tile_pool(name="small", bufs=1) as small,
            tc.tile_pool(name="sm2", bufs=4) as sm2,
            tc.tile_pool(name="tmp", bufs=1) as tmpp,       # (128, POS) bf16 acc
            tc.tile_pool(name="tmb", bufs=2) as tmbp,       # (128, POS//2) bf16
            tc.tile_pool(name="outp", bufs=3) as outp,      # (128, 512) f32
            tc.tile_pool(name="psum", bufs=8, space="PSUM") as psp,
            tc.tile_pool(name="dram", bufs=1, space="DRAM") as dramp,
        ):
            # ---- weights / consts ----
            w1t = []
            for q in range(NCHUNK):
                t_ = consts.tile([CPK, K], bf16, tag=f"w1t{q}")
                nc.sync.dma_start(t_, w1t_d[:][q * CPK:(q + 1) * CPK, :])
                w1t.append(t_)
            b1 = consts.tile([K, 1], f32, tag="b1")
            nc.sync.dma_start(b1, b1_d[:])
            wpt = consts.tile([K, 4 * 128], f32, tag="wpt")
            nc.sync.dma_start(wpt, wpt_d[:])
            bp = consts.tile([T, 128], f32, tag="bp")
            nc.sync.dma_start(bp, bp_d[:])
            wqt = consts.tile([K, 4 * 128], f32, tag="wqt")
            nc.sync.dma_start(wqt, wqt_d[:])
            bq = consts.tile([T, 128], f32, tag="bq")
            nc.sync.dma_start(bq, bq_d[:])
            wtt = consts.tile([K, 2 * 32], f32, tag="wtt")
            nc.sync.dma_start(wtt, wtt_d[:])
            bt = consts.tile([1, 32], f32, tag="bt")
            nc.sync.dma_start(bt, bt_d[:])
            w2t = []
            for q in range(NCHUNK):
                t_ = consts.tile([CPK, C], bf16, tag=f"w2t{q}")
                nc.sync.dma_start(t_, w2t_d[:][q * CPK:(q + 1) * CPK, :])
                w2t.append(t_)
            b2 = []
            for oc in range(NCHUNK):
                t_ = consts.tile([CPK, 1], f32, tag=f"b2_{oc}")
                nc.sync.dma_start(t_, b2_d[:][oc * CPK:(oc + 1) * CPK, :])
                b2.append(t_)

            # ---- load x, convert to bf16 ----
            xbf = []
            for q in range(NCHUNK):
                xb = big.tile([CPK, POS], bf16, tag="big")
                for h in range(4):
                    xf = stage.tile([CPK, POS // 4], f32, tag="stage")
                    nc.sync.dma_start(
                        xf, x_d[:][q * CPK:(q + 1) * CPK,
                                   h * (POS // 4):(h + 1) * (POS // 4)])
                    nc.scalar.activation(
                        xb[:, h * (POS // 4):(h + 1) * (POS // 4)], xf,
                        AF.Copy)
                xbf.append(xb)

            # ---- conv1 + BN + ReLU -> o (K, POS) bf16 ----
            o_sb = small.tile([K, POS], bf16, tag="o")
            for nt in range(16):
                ps = psp.tile([K, 512], f32, tag="ps")
                for q in range(NCHUNK):
                    nc.tensor.matmul(
                        ps, w1t[q], xbf[q][:, nt * 512:(nt + 1) * 512],
                        start=(q == 0), stop=(q == NCHUNK - 1))
                nc.scalar.activation(
                    o_sb[:, nt * 512:(nt + 1) * 512], ps, AF.Relu,
                    bias=b1, scale=1.0)

            # ---- pools ----
            rp = sm2.tile([K, T * S], f32, tag="rp")   # (8, 32) [t,a]
            nc.vector.tensor_reduce(
                rp, o_sb.rearrange("p (g r) -> p g r", r=HB * W),
                AX.X, ALU.max)
            cp = sm2.tile([K, T * S], f32, tag="cp")   # (8, 32) [t,b]
            nc.vector.tensor_reduce(
                cp, o_sb.rearrange("p (t h b v) -> p t b h v", t=T, h=H, b=S),
                AX.XY, ALU.max)
            tp = sm2.tile([K, TS], f32, tag="tp")      # (8, 2)
            nc.vector.tensor_reduce(
                tp, o_sb.rearrange("p (g r) -> p g r", g=TS),
                AX.X, ALU.add)

            # ---- p/q/tm small matmuls + softmax (fp32) ----
            def softmax_rowgroups(name, ps_tile, bias_tile, parts, ngroups, gsz):
                """exp/sum/recip/mult over contiguous groups of gsz."""
                raw = sm2.tile([parts, ngroups * gsz], f32, tag=f"{name}_raw")
                nc.vector.tensor_tensor(raw, ps_tile, bias_tile, ALU.add)
                ex = sm2.tile([parts, ngroups * gsz], f32, tag=f"{name}_ex")
                nc.scalar.activation(ex, raw, AF.Exp)
                ss = sm2.tile([parts, ngroups], f32, tag=f"{name}_ss")
                nc.vector.tensor_reduce(
                    ss, ex.rearrange("p (g r) -> p g r", r=gsz), AX.X, ALU.add)
                rr = sm2.tile([parts, ngroups], f32, tag=f"{name}_rr")
                nc.vector.reciprocal(rr, ss)
                sm = sm2.tile([parts, ngroups * gsz], f32, tag=f"{name}_sm")
                for b in range(gsz):
                    nc.vector.tensor_tensor(
                        sm.rearrange("p (g r) -> p g r", r=gsz)[:, :, b],
                        ex.rearrange("p (g r) -> p g r", r=gsz)[:, :, b],
                        rr, ALU.mult)
                return sm

            ps_p = psp.tile([T, 128], f32, tag="ps")
            for i in range(4):
                nc.tensor.matmul(
                    ps_p, rp.rearrange("p (t a) -> p a t", a=S)[:, i, :],
                    wpt[:, i * 128:(i + 1) * 128],
                    start=(i == 0), stop=(i == 3))
            p_sm = softmax_rowgroups("p", ps_p, bp, T, 32, 4)

            ps_q = psp.tile([T, 128], f32, tag="ps")
            for i in range(4):
                nc.tensor.matmul(
                    ps_q, cp.rearrange("p (t b) -> p b t", b=S)[:, i, :],
                    wqt[:, i * 128:(i + 1) * 128],
                    start=(i == 0), stop=(i == 3))
            q_sm = softmax_rowgroups("q", ps_q, bq, T, 32, 4)

            ps_t = psp.tile([1, 32], f32, tag="ps")
            for i in range(2):
                nc.tensor.matmul(
                    ps_t, tp[:, i:i + 1], wtt[:, i * 32:(i + 1) * 32],
                    start=(i == 0), stop=(i == 1))
            t_sm = softmax_rowgroups("t", ps_t, bt, 1, 16, 2)

            # ---- round-trip through DRAM to build per-channel scalar tables ----
            p_scr = dramp.tile([T, 128], f32, tag="p_scr")
            q_scr = dramp.tile([T, 128], f32, tag="q_scr")
            t_scr = dramp.tile([1, 32], f32, tag="t_scr")
            nc.gpsimd.dma_start(p_scr, p_sm)
            nc.gpsimd.dma_start(q_scr, q_sm)
            nc.gpsimd.dma_start(t_scr, t_sm)

            def bcast_tbl(name, scr, ncols, q):
                """(128, T*ncols or ncols) table: row c <- values for k-group
                of channel q*128+c. scr is DRAM (T, 128)[t, k*16+j] for p/q
                (ncols=16), or (1, 32)[k*4+j] for tm (ncols=4)."""
                per_t = scr.shape[0] == T
                width = (T if per_t else 1) * ncols
                tbl = consts.tile([CPK, width], f32, tag=f"{name}{q}")
                for half in range(2):
                    kk = 2 * q + half
                    if per_t:
                        in_ap = bass.AP(
                            tensor=tbl.tensor if False else scr.tensor,
                            offset=scr.offset + kk * ncols,
                            ap=[[0, 64], [128, T], [1, ncols]])
                        out_ap = tbl[half * 64:(half + 1) * 64, :].rearrange(
                            "p (t g) -> p t g", g=ncols)
                    else:
                        in_ap = bass.AP(
                            tensor=scr.tensor,
                            offset=scr.offset + kk * ncols,
                            ap=[[0, 64], [1, ncols]])
                        out_ap = tbl[half * 64:(half + 1) * 64, :]
                    nc.gpsimd.dma_start(out_ap, in_ap)
                return tbl

            # ---- spatial transform per chunk (bf16) ----
            ytm = []
            for q in range(NCHUNK):
                P_bc = bcast_tbl("Pbc", p_scr, 16, q)   # (128, [t,a,b])
                Q_bc = bcast_tbl("Qbc", q_scr, 16, q)   # (128, [t,b,a])
                T_bc = bcast_tbl("Tbc", t_scr, 4, q)    # (128, [a2,b2])

                xq = xbf[q]
                y1 = big.tile([CPK, POS], bf16, tag="big")
                accb = tmpp.tile([CPK, POS], bf16, tag="acc",
                                 name=f"accb_{q}")
                # Q-pass: mix w-blocks (stride-8 groups of v within each h).
                # Terms outermost so consecutive DVE ops are independent.
                def qblk(src, t, j):
                    return bass.AP(
                        tensor=src.tensor,
                        offset=src.offset + t * 1024 + j * WB,
                        ap=[list(src.ap[0]), [W, H], [1, WB]])

                def ablk(t, bw):
                    i = t * 4 + bw
                    return accb[:, i * 256:(i + 1) * 256].rearrange(
                        "p (h v) -> p h v", v=WB)

                for aw in range(S):
                    for t in range(T):
                        for bw in range(S):
                            scal = Q_bc[:, t * 16 + bw * 4 + aw:
                                        t * 16 + bw * 4 + aw + 1]
                            if aw == 0:
                                nc.vector.tensor_scalar_mul(
                                    ablk(t, bw), qblk(xq, t, aw), scal)
                            elif aw < S - 1:
                                nc.vector.scalar_tensor_tensor(
                                    ablk(t, bw), qblk(xq, t, aw), scal,
                                    ablk(t, bw), ALU.mult, ALU.add)
                            else:
                                nc.vector.scalar_tensor_tensor(
                                    qblk(y1, t, bw), qblk(xq, t, aw), scal,
                                    ablk(t, bw), ALU.mult, ALU.add)
                # P-pass: mix h-blocks (contiguous 256-blocks), y1 -> xq buffer
                def pblk(src, t, j):
                    return src[:, t * 1024 + j * 256:t * 1024 + j * 256 + 256]

                def ablk2(t, ah):
                    i = t * 4 + ah
                    return accb[:, i * 256:(i + 1) * 256]

                for bh in range(S):
                    for t in range(T):
                        for ah in range(S):
                            scal = P_bc[:, t * 16 + ah * 4 + bh:
                                        t * 16 + ah * 4 + bh + 1]
                            if bh == 0:
                                nc.vector.tensor_scalar_mul(
                                    ablk2(t, ah), pblk(y1, t, bh), scal)
                            elif bh < S - 1:
                                nc.vector.scalar_tensor_tensor(
                                    ablk2(t, ah), pblk(y1, t, bh), scal,
                                    ablk2(t, ah), ALU.mult, ALU.add)
                            else:
                                nc.vector.scalar_tensor_tensor(
                                    pblk(xq, t, ah), pblk(y1, t, bh), scal,
                                    ablk2(t, ah), ALU.mult, ALU.add)
                # TM-pass: mix t-halves, xq -> y1 (y1 becomes y_tm)
                HALF = POS // 2
                for a2 in range(TS):
                    tmb = tmbp.tile([CPK, HALF], bf16, tag="tmb")
                    nc.vector.tensor_scalar_mul(
                        tmb, xq[:, 0:HALF], T_bc[:, a2 * 2:a2 * 2 + 1])
                    nc.vector.scalar_tensor_tensor(
                        y1[:, a2 * HALF:(a2 + 1) * HALF],
                        xq[:, HALF:POS], T_bc[:, a2 * 2 + 1:a2 * 2 + 2],
                        tmb, ALU.mult, ALU.add)
                ytm.append(y1)

            # ---- conv2 + BN + ReLU -> y ----
            for oc in range(NCHUNK):
                for ntg in range(2):
                    pss = [psp.tile([CPK, 512], f32, tag="ps",
                                    name=f"ps_c2_{oc}_{ntg}_{j}")
                           for j in range(8)]
                    for cq in range(NCHUNK):
                        for j in range(8):
                            nt = ntg * 8 + j
                            nc.tensor.matmul(
                                pss[j],
                                w2t[cq][:, oc * CPK:(oc + 1) * CPK],
                                ytm[cq][:, nt * 512:(nt + 1) * 512],
                                start=(cq == 0), stop=(cq == NCHUNK - 1))
                    for j in range(8):
                        nt = ntg * 8 + j
                        ob = outp.tile([CPK, 512], f32, tag="ob")
                        nc.scalar.activation(ob, pss[j], AF.Relu,
                                             bias=b2[oc], scale=1.0)
                        nc.sync.dma_start(
                            y_d[:][oc * CPK:(oc + 1) * CPK,
                                   nt * 512:(nt + 1) * 512], ob)

    _split_waits(nc, mybir)
    return nc


def _host_prep(inputs):
    """Fold BN into conv weights, build device-layout weight arrays."""
    f = np.float32
    conv1_w = np.asarray(inputs["conv1_w"], f)
    conv1_b = np.asarray(inputs["conv1_b"], f)
    s1 = np.asarray(inputs["bn1_g"], f) / np.sqrt(np.asarray(inputs["bn1_v"], f) + EPS)
    w1 = conv1_w * s1[:, None]
    b1 = (conv1_b - np.asarray(inputs["bn1_m"], f)) * s1 + np.asarray(inputs["bn1_b"], f)

    convp_w = np.asarray(inputs["convp_w"], f)   # (128, 8, 4)
    convq_w = np.asarray(inputs["convq_w"], f)
    convt_w = np.asarray(inputs["convt_w"], f)   # (32, 8, 2)

    wpt = np.transpose(convp_w, (1, 2, 0)).reshape(K, 4 * 128).copy()
    bp = np.tile(np.asarray(inputs["convp_b"], f)[None, :], (T, 1))

    # permute q outputs o=k*16+a*4+b -> o'=k*16+b*4+a  (a innermost for softmax)
    perm = np.arange(128).reshape(8, 4, 4).transpose(0, 2, 1).reshape(128)
    wq_p = convq_w[perm]                          # row o' = orig row (k,a,b)
    bq_p = np.asarray(inputs["convq_b"], f)[perm]
    wqt = np.transpose(wq_p, (1, 2, 0)).reshape(K, 4 * 128).copy()
    bq = np.tile(bq_p[None, :], (T, 1))

    wtt = (np.transpose(convt_w, (1, 2, 0)) / (TB * H * W)).reshape(K, 2 * 32).copy()
    bt = np.asarray(inputs["convt_b"], f).reshape(1, 32)

    conv2_w = np.asarray(inputs["conv2_w"], f)
    s2 = np.asarray(inputs["bn2_g"], f) / np.sqrt(np.asarray(inputs["bn2_v"], f) + EPS)
    w2 = conv2_w * s2[:, None]
    b2 = (np.asarray(inputs["conv2_b"], f) - np.asarray(inputs["bn2_m"], f)) * s2 \
        + np.asarray(inputs["bn2_b"], f)

    bf = ml_dtypes.bfloat16
    return {
        "w1t": np.ascontiguousarray(w1.T).astype(bf),
        "b1": b1.reshape(K, 1).copy(),
        "wpt": wpt, "bp": bp.copy(),
        "wqt": wqt, "bq": bq.copy(),
        "wtt": wtt, "bt": bt.copy(),
        "w2t": np.ascontiguousarray(w2.T).astype(bf),
        "b2": b2.reshape(C, 1).copy(),
    }


def kernel(**inputs) -> np.ndarray:
    from concourse.bass_utils import run_bass_kernel_spmd

    if "nc" not in _CACHE:
        _CACHE["nc"] = _build()
    nc = _CACHE["nc"]

    shared = _host_prep(inputs)
    x = np.asarray(inputs["x"], np.float32)       # (8, 512, 8, 32, 32)
    in_maps = []
    for i in range(NCORES):
        m = dict(shared)
        m["x"] = np.ascontiguousarray(x[i].reshape(C, POS))
        in_maps.append(m)

    res = run_bass_kernel_spmd(nc, in_maps, list(range(NCORES)))
    out = np.stack([res.results[i]["y"].reshape(C, T, H, W)
                    for i in range(NCORES)])
    return out.astype(np.float32)
